# revision 90
# baseline (speedup 1.0000x reference)
"""Trainium2 Bass kernel for MultiHeadCrossAttention.

Problem shapes (hardcoded; see module constants):
  query      [8, 512, 768] f32
  key_value  [8, 2048, 768] f32
  kv_mask    [8, 2048] bool
  Wq/Wk/Wv   [768, 1024] f32, Wo [1024, 1024] f32, biases [1024] f32

Sharding: pure data-parallel -- batch element b runs on core b (8 cores, no
collectives). Each core computes the full attention stack for its batch
element and writes out^T [1024, 512]; the host transposes and stacks.

Host-side prep (as the 140.2us baseline): kv-mask compaction to NKV*128
rows, bf16 weights/activations, kv_mask folded to an additive bias, bv
folded into bo, q/kv pre-transposed, Wq/Wk packed per-column-block.

v2 changes over the 140.2us baseline (cost-model time now ~133.3us):
  - All per-partition bias vectors (bq|bk|bo_eff|mask) are pre-TRANSPOSED
    on the host and appended to the q DMA as bf16 columns -- no separate
    bias DMA, no identity-matrix dependency, no PE transpose at startup
    (one DVE copy widens them to f32 for the tensor_scalar ops).
  - kv^T arrives in independent piece TILES (one DMA each) so the
    transfers pipeline with no WAW serialization, and the load order makes
    every pair-0 input an early item on its queue (gpsimd wq0+wk0 via
    SWDGE, scalar kv pieces, sync q+bias then a 256-col Wv slice for
    heads 0-3). S(0,0) fires at ~8us instead of ~11.5us; V projection is
    emitted in 4-head column groups so pair 0 only waits the small Wv
    slice. K-proj pieces align to the kv piece tiles.
  - K-proj is trimmed to the exact compacted kv count MKV (KT pad cols are
    memset once; their scores die under the -30000 mask bias).
  - O^T transposes for pairs 0-5 go to the DMA crossbar (dma_start_transpose
    on sync) instead of the PE: part_a writes a [128,qb,sub,dh] interleave,
    4 [128,128] xbar tiles write OT directly (no otps PSUM, no part_b).
    Pair 6 keeps the PE path (its OT feeds outB1 too soon for the xbar
    latency); pair 7's normalize runs inline at the tail.
  - Tail: while the pair-7 normalize runs on DVE/Act, the idle PE preloads
    outpart[mt] into psum banks via identity matmuls that OPEN the
    accumulation groups (start=True); the OT[7] matmuls then accumulate
    onto them (start=False, stop=True), so the final op per block is a
    cheap psum->sbuf cast copy alternating DVE/Act, and the output leaves
    as 4 two-block DMAs alternating sync/scalar. (GPSIMD cannot touch
    PSUM on hardware, and matmul-accumulate over engine-written PSUM
    mis-lowers on hardware -- hence the identity-matmul preloads.)
  - The cost model locks each matmul's p-state price when it enters the
    PE exec queue, so the schedule keeps ~30 dummy warmup matmuls and the
    _TUNE knobs were picked by timeline-sim sweep with numerics re-checked
    in CoreSim (a faster schedule that drops work is worthless).
"""

import numpy as np
import ml_dtypes

import concourse.bass as bass
import concourse.bacc as bacc
import concourse.mybir as mybir
import concourse.tile as tile
from concourse.bass_utils import run_bass_kernel_spmd

dt = mybir.dt
AF = mybir.ActivationFunctionType

B = 8
LQ = 512
LKV = 2048
QD = 768
HID = 1024
H = 16
DH = 64
SCALE = DH**-0.5
MASK_NEG = -30000.0

F32 = dt.float32
BF16 = dt.bfloat16

NQT = QD // 128  # 6 feature tiles
NH = HID // 128  # 8 hidden tiles

# schedule tuning knobs (defaults chosen by timeline-sim sweep)
_TUNE = {
    "warmup": 30,        # dummy p-state ramp matmuls
    "kv_cuts": (0, 256, 512, 768),  # kv piece boundaries
    "pv_lag0": 1,        # extra PV lag for pair 0
    "fill0_pops": 2,     # pair-0 fill units consumed per kc slot
    "qt_slot": 1,        # slot for the t+2 Q projection
    "fillC_from": 2,     # first pair-7 slot consuming outB1 units
}


def build_nc(NKV, MKV):
    """Per-core kernel; compacted kv capacity NKV*128 rows, exact max
    unmasked count MKV (K-proj only computes cols < MKV)."""
    LKVC = NKV * 128
    MKV = min(MKV, LKVC)
    NBC = 3 * NH + NKV  # bias cols appended to q: bq|bk|bo_eff|mask

    # kv arrives in 3 independent piece TILES (separate tiles so the DMAs
    # pipeline with no WAW serialization); K-proj pieces align to them.
    kv_cuts = [c for c in _TUNE["kv_cuts"] if c < LKVC] + [LKVC]
    kv_loads = list(zip(kv_cuts[:-1], kv_cuts[1:]))

    def k_pieces():
        # <=512-col pieces, each inside one kv piece tile
        pieces = []
        for p0, p1 in kv_loads:
            c = p0
            hi = min(p1, MKV)
            while c < hi:
                pieces.append((c, min(c + 512, hi)))
                c = min(c + 512, hi)
        return pieces

    nc = bacc.Bacc("TRN2", target_bir_lowering=False, debug=False)

    q_d = nc.dram_tensor(
        "q_pk", [128, NQT * LQ + NBC], BF16, kind="ExternalInput"
    )
    kv_d = nc.dram_tensor("kv_pk", [128, NQT * LKVC], BF16, kind="ExternalInput")
    wq_d = nc.dram_tensor("Wq_pk", [NH, 128, QD], BF16, kind="ExternalInput")
    wk_d = nc.dram_tensor("Wk_pk", [NH, 128, QD], BF16, kind="ExternalInput")
    wv_d = nc.dram_tensor("Wv_bf", [QD, HID], BF16, kind="ExternalInput")
    wo_d = nc.dram_tensor("Wo_bf", [HID, HID], BF16, kind="ExternalInput")
    id_d = nc.dram_tensor("ident", [128, 128], BF16, kind="ExternalInput")
    out_d = nc.dram_tensor("out", [HID, LQ], BF16, kind="ExternalOutput")

    with tile.TileContext(nc) as tc:
        with (
            tc.tile_pool(name="persist", bufs=1) as persist,
            tc.tile_pool(name="ppool", bufs=3) as ppool,
            tc.tile_pool(name="nrm", bufs=2) as nrm,
            tc.tile_pool(name="finpool", bufs=8) as finpool,
            # PSUM: 8 banks of 2KB/partition total.
            tc.tile_pool(name="sps", bufs=3, space="PSUM") as sps,    # 3 banks
            tc.tile_pool(name="proj", bufs=2, space="PSUM") as proj,  # 2 banks
            tc.tile_pool(name="ops", bufs=3, space="PSUM") as ops,    # 3 banks
        ):
            qb_t = persist.tile([128, NQT * LQ + NBC], BF16, tag="qTb", name="qTb")

            def qTb(kt):
                return qb_t[:, kt * LQ : (kt + 1) * LQ]
            # bias columns ride the q DMA as bf16; widen to f32 once (the
            # tensor_scalar ops require an f32 scalar operand)
            bias_bf = qb_t[:, NQT * LQ :]
            bc32 = persist.tile([128, NBC], F32, tag="bc32", name="bc32")
            bq_sb = bc32[:, 0:NH]
            bk_sb = bc32[:, NH : 2 * NH]
            boe_sb = bc32[:, 2 * NH : 3 * NH]
            mb_sb = bc32[:, 3 * NH : 3 * NH + NKV]
            kvt = [
                persist.tile(
                    [128, NQT, c1 - c0], BF16, tag=f"kvt{i}", name=f"kvt{i}"
                )
                for i, (c0, c1) in enumerate(kv_loads)
            ]

            def kv_ap(kt, c0, c1):
                """SBUF AP for kv^T cols [c0, c1) (must lie in one piece)."""
                for (p0, p1), tile_ in zip(kv_loads, kvt):
                    if c0 >= p0 and c1 <= p1:
                        return tile_[:, kt, c0 - p0 : c1 - p0]
                raise AssertionError(f"kv range {c0}:{c1} spans pieces")

            ident = persist.tile([128, 128], BF16, tag="ident")
            wqb = [None] * NH
            wkb = [None] * NH
            # Wv column groups: wvg[0] = hid cols 0:256 (heads 0-3) from its
            # own small DMA; cols 256:1024 (heads 4-15) in one tile.
            wv0_t = persist.tile([128, NQT, 256], BF16, tag="wv0", name="wv0")
            wvr_t = persist.tile([128, NQT, 768], BF16, tag="wvr", name="wvr")
            wo_bf = []

            def load_wblock(dst_list, src_d, b, eng):
                wt = persist.tile(
                    [128, NQT, 128], BF16, tag=f"w{id(src_d)}b{b}",
                    name=f"wb{b}",
                )
                eng.dma_start(wt.rearrange("p kt c -> p (kt c)"), src_d[b])
                dst_list[b] = wt

            def load_wv_cols(dst, c0, c1, eng):
                eng.dma_start(
                    dst[:],
                    wv_d.ap()[:, c0:c1].rearrange("(kt p) c -> p kt c", p=128),
                )

            def load_kv_piece(i, eng):
                c0, c1 = kv_loads[i]
                eng.dma_start(
                    kvt[i][:],
                    kv_d.ap().rearrange("p (f n) -> p f n", n=LKVC)[:, :, c0:c1],
                )

            # ---- staging DMAs ---------------------------------------------
            # All transfers serialize on the shared DMA-engine device in
            # ~emission order (independent tiles pipeline; same-tile pieces
            # would serialize on WAW sems). Priority: q+bias, kv piece 0,
            # wk0/wq0 (SWDGE, parallel gen), kv piece 1, wv heads 0-3, wq1,
            # kv piece 2, wk1, wv rest, then the remaining weights.
            load_wblock(wqb, wq_d, 0, nc.gpsimd)
            load_kv_piece(0, nc.scalar)
            nc.sync.dma_start(qb_t[:], q_d[:])
            load_wblock(wkb, wk_d, 0, nc.gpsimd)
            for i in range(1, len(kv_loads)):
                load_kv_piece(i, nc.scalar)
            load_wv_cols(wv0_t, 0, 256, nc.sync)
            load_wblock(wqb, wq_d, 1, nc.gpsimd)
            load_wblock(wkb, wk_d, 1, nc.gpsimd)
            load_wv_cols(wvr_t, 256, 1024, nc.sync)
            for b in range(2, NH):
                load_wblock(wkb, wk_d, b, nc.scalar)
                load_wblock(wqb, wq_d, b, nc.gpsimd)
            nc.gpsimd.dma_start(ident[:], id_d[:])
            for kt in range(NH):
                wt = persist.tile([128, HID], BF16, tag=f"wo{kt}", name=f"wo{kt}")
                nc.sync.dma_start(wt[:], wo_d[kt * 128 : (kt + 1) * 128, :])
                wo_bf.append(wt)

            # PE p-state warmup on dummy matmuls of a memset scratch tile
            # (proven recipe: ~3.2us of continuous dummy work from t~1us
            # leaves every later matmul at full clock).
            wsc = persist.tile([128, 128], BF16, tag="wsc", name="wsc")
            nc.vector.memset(wsc[:], 0.0)
            wrm = None
            for i in range(_TUNE["warmup"]):
                if i % 4 == 0:
                    wrm = sps.tile([128, 512], F32, tag="sps", name="wrm")
                nc.tensor.matmul(
                    wrm[:, (i % 4) * 128 : (i % 4 + 1) * 128],
                    wsc[:],
                    wsc[:],
                    start=True,
                    stop=True,
                )

            # KT pad columns [MKV:LKVC] are read by the last kv chunk's S
            # matmuls but never written by the trimmed K-proj; zero once.
            KT = [
                persist.tile([128, LKVC], BF16, tag=f"KT{t}", name=f"KT{t}")
                for t in range(NH)
            ]
            if MKV < LKVC:
                for t in range(NH):
                    nc.vector.memset(KT[t][:, MKV:LKVC], 0.0)
            nc.vector.tensor_copy(bc32[:], bias_bf)

            # ---- Q^T projection tiles [128, 512] bf16 ---------------------
            QT = [None] * NH

            def emit_qtproj(mt):
                ps = proj.tile([128, 512], F32, tag="proj", name="ps")
                for kt in range(NQT):
                    nc.tensor.matmul(
                        ps[:],
                        wqb[mt][:, kt, :],
                        qTb(kt),
                        start=(kt == 0),
                        stop=(kt == NQT - 1),
                    )
                qt_t = persist.tile([128, LQ], BF16, tag=f"QT{mt}", name=f"QT{mt}")
                nc.vector.tensor_scalar_add(qt_t[:], ps[:], bq_sb[:, mt : mt + 1])
                QT[mt] = qt_t

            # ---- V projection in 4-head column groups ---------------------
            # V_il[lt] is [128, 16, 65] head-interleaved with a ones column
            # (PV's extra column = softmax denominator). Group g covers
            # heads 4g..4g+3 (pairs 2g, 2g+1).
            V_il = [None] * NKV

            def emit_vproj_g(lt, g):
                if V_il[lt] is None:
                    vt = persist.tile(
                        [128, H, DH + 1], BF16, tag=f"V{lt}", name=f"V{lt}"
                    )
                    nc.vector.memset(vt[:, :, DH], 1.0)
                    V_il[lt] = vt
                vt = V_il[lt]
                wsrc = wv0_t if g == 0 else wvr_t
                coff = 0 if g == 0 else (g - 1) * 256
                ps = proj.tile([128, 512], F32, tag="proj", name="ps")
                for kt in range(NQT):
                    nc.tensor.matmul(
                        ps[:, 0:256],
                        kv_ap(kt, lt * 128, (lt + 1) * 128),
                        wsrc[:, kt, coff : coff + 256],
                        start=(kt == 0),
                        stop=(kt == NQT - 1),
                    )
                nc.vector.tensor_copy(
                    vt[:, 4 * g : 4 * g + 4, 0:DH],
                    ps[:, 0:256].rearrange("p (h d) -> p h d", d=DH),
                )

            # ---- K^T projection (trimmed to MKV cols) ---------------------
            def emit_ktproj(t, c0, c1):
                w = c1 - c0
                ps = proj.tile([128, 512], F32, tag="proj", name="ps")
                for kt in range(NQT):
                    nc.tensor.matmul(
                        ps[:, 0:w],
                        wkb[t][:, kt, :],
                        kv_ap(kt, c0, c1),
                        start=(kt == 0),
                        stop=(kt == NQT - 1),
                    )
                nc.vector.tensor_scalar_add(
                    KT[t][:, c0:c1], ps[:, 0:w], bk_sb[:, t : t + 1]
                )

            kp0 = k_pieces()
            kpN = k_pieces()
            emit_qtproj(0)
            emit_ktproj(0, *kp0[0])

            # Pair-0 PE fill queue in expected data-arrival order: each
            # K-proj piece, the V group-0 chunks its kv cols enable, then
            # pair-1 projections.
            fill0 = []

            def mk_v(j, g):
                return lambda: emit_vproj_g(j, g)

            nv = 0
            for pi, (c0, c1) in enumerate(kp0[1:], start=1):
                fill0.append(lambda c0=c0, c1=c1: emit_ktproj(0, c0, c1))
                if pi == 1 and NH > 1:
                    fill0.append(lambda: emit_qtproj(1))
                vmax = min(-(-c1 // 128), NKV)
                while nv < vmax:
                    fill0.append(mk_v(nv, 0))
                    nv += 1
            while nv < NKV:
                fill0.append(mk_v(nv, 0))
                nv += 1
            if NH > 2:
                fill0.append(lambda: emit_qtproj(2))
            for c0, c1 in kpN:
                if NH > 1:
                    fill0.append(lambda c0=c0, c1=c1: emit_ktproj(1, c0, c1))
            # V groups 1-3 (heads 4-15): fill for pairs 1-3; group g first
            # needed by pair 2g slot 1.
            fillB = [mk_v(j, g) for g in range(1, 4) for j in range(NKV)]

            # ---- attention per head pair ---------------------------------
            outpart = [None] * NH

            def emit_outA(mt):
                ps = proj.tile([128, 512], F32, tag="proj", name="ps")
                for kt in range(4):
                    nc.tensor.matmul(
                        ps[:],
                        wo_bf[kt][:, mt * 128 : (mt + 1) * 128],
                        OT[kt][:],
                        start=(kt == 0),
                        stop=(kt == 3),
                    )
                op_t = persist.tile(
                    [128, 512], BF16, tag=f"outpart{mt}", name=f"outpart{mt}"
                )
                with nc.allow_low_precision(reason="bf16 out partial"):
                    nc.vector.tensor_scalar_add(
                        op_t[:], ps[:], boe_sb[:, mt : mt + 1]
                    )
                outpart[mt] = op_t

            kt_slots = {}
            for ci in range(len(kpN)):
                kt_slots[max(1, ((ci + 1) * NKV) // (len(kpN) + 1))] = ci
            assert len(kt_slots) == len(kpN), "K-proj piece slot collision"
            qt_slot = min(_TUNE["qt_slot"], NKV - 1)
            # outA: OT[3]'s xbar lands ~2 slots into pair 4.
            oa_pair = {4: [0, 1], 5: [2, 3], 6: [4, 5, 6, 7]}
            oa_slots = {
                4: {min(4, NKV - 2): 0, min(7, NKV - 1): 1},
                5: {min(2, NKV - 2): 2, min(5, NKV - 1): 3},
                6: {
                    min(2, NKV - 4): 4,
                    min(4, NKV - 3): 5,
                    min(6, NKV - 2): 6,
                    min(8, NKV - 1): 7,
                },
            }

            def emit_outB1(mt, eng=None, pool=None, tag="proj"):
                ps = (pool or proj).tile([128, 512], F32, tag=tag, name="ps")
                for kt in range(4, NH - 1):
                    nc.tensor.matmul(
                        ps[:],
                        wo_bf[kt][:, mt * 128 : (mt + 1) * 128],
                        OT[kt][:],
                        start=(kt == 4),
                        stop=(kt == NH - 2),
                    )
                with nc.allow_low_precision(reason="bf16 out partial"):
                    (eng or nc.vector).tensor_tensor(
                        outpart[mt][:], outpart[mt][:], ps[:],
                        mybir.AluOpType.add,
                    )

            fillC = [
                lambda eng=None, mt=mt, **kw: emit_outB1(mt, eng, **kw)
                for mt in range(NH)
            ]

            def make_norm_xbar(t, o_ps):
                """Deferred normalize via the DMA crossbar (sync queue)."""
                ot_t = persist.tile([128, LQ], BF16, tag=f"KT{t}", name=f"OT{t}")
                nobx = nrm.tile(
                    [128, 4, 2, DH], BF16, tag="nobx", name="nobx", bufs=2
                )

                def part_a():
                    for sub in range(2):
                        opv = o_ps[sub].rearrange("p (q c) -> p q c", c=DH + 1)
                        rc = nrm.tile([128, 4, 1], F32, tag="rc", name="rc")
                        nc.vector.reciprocal(rc[:], opv[:, :, DH : DH + 1])
                        nc.vector.tensor_tensor(
                            nobx[:, :, sub, :],
                            opv[:, :, 0:DH],
                            rc[:].broadcast_to([128, 4, DH]),
                            mybir.AluOpType.mult,
                        )

                def part_b():
                    for qb in range(4):
                        nc.sync.dma_start_transpose(
                            ot_t[:, qb * 128 : (qb + 1) * 128],
                            nobx[:, qb, :, :],
                        )
                    OT[t] = ot_t

                return [part_a, part_b]

            def make_norm_pe(t, o_ps):
                """PE-transpose normalize for pair 6 (OT[6] is consumed too
                soon into pair 7 for the xbar latency)."""
                ot_t = persist.tile([128, LQ], BF16, tag=f"KT{t}", name=f"OT{t}")
                otps = ops.tile([64, 1024], BF16, tag="ops", name="otps")

                def part_a():
                    nobs = []
                    for sub in range(2):
                        opv = o_ps[sub].rearrange("p (q c) -> p q c", c=DH + 1)
                        rc = nrm.tile([128, 4, 1], F32, tag="rc", name="rc")
                        nc.vector.reciprocal(rc[:], opv[:, :, DH : DH + 1])
                        nob = nrm.tile(
                            [128, 4, DH], BF16, tag="nob", name="nob", bufs=3
                        )
                        nc.vector.tensor_tensor(
                            nob[:],
                            opv[:, :, 0:DH],
                            rc[:].broadcast_to([128, 4, DH]),
                            mybir.AluOpType.mult,
                        )
                        nobs.append(nob)
                    for g in range(8):
                        nc.tensor.transpose(
                            otps[:, g * 128 : (g + 1) * 128],
                            nobs[g // 4][:, g % 4, :],
                            ident[:],
                        )

                def part_b():
                    for sub in range(2):
                        nc.vector.tensor_copy(
                            ot_t[sub * 64 : sub * 64 + 64, :],
                            otps[:, sub * 512 : (sub + 1) * 512],
                        )
                    OT[t] = ot_t

                return [part_a, part_b]

            OT = [None] * NH
            pending_norm = []
            for t in range(NH):
                o_ps = [
                    ops.tile([128, 4 * (DH + 1)], F32, tag="ops", name="o_ps")
                    for _ in range(2)
                ]
                # pair 0's V group lands ~1 slot later than its first PV
                # would like; lag the PV stream one extra slot there.
                pv_lag = _TUNE["pv_lag0"] if t == 0 else 1
                p_hist = []
                for kc in range(NKV + pv_lag):
                    if kc < NKV:
                        p_cur = [None, None]
                        for sub in range(2):
                            off = sub * 64
                            s = sps.tile([128, 512], F32, tag="sps", name="s")
                            nc.tensor.matmul(
                                s[:],
                                KT[t][off : off + 64, kc * 128 : (kc + 1) * 128],
                                QT[t][off : off + 64, :],
                                start=True,
                                stop=True,
                            )
                            p = ppool.tile(
                                [128, 512], BF16, tag="p", name="p", bufs=6
                            )
                            nc.scalar.activation(
                                p[:], s[:], AF.Exp,
                                bias=mb_sb[:, kc : kc + 1], scale=SCALE,
                            )
                            p_cur[sub] = p
                        p_hist.append(p_cur)
                    if kc < len(pending_norm):
                        pending_norm[kc]()

                    def emit_pv():
                        kcp = kc - pv_lag
                        pp = p_hist[kcp]
                        for sub in range(2):
                            for qb in range(4):
                                nc.tensor.matmul(
                                    o_ps[sub][:, qb * 65 : qb * 65 + 65],
                                    pp[sub][:, qb * 128 : (qb + 1) * 128],
                                    V_il[kcp][:, 2 * t + sub, :],
                                    start=(kcp == 0 and qb == 0),
                                    stop=(kcp == NKV - 1 and qb == 3),
                                )

                    # For the LAST pair the S/PV chain goes AHEAD of the
                    # fills: fills otherwise delay the final S matmuls in
                    # the in-order PE queue, starving the Act exp stream
                    # whose last exp gates the whole tail.
                    if t == NH - 1 and kc >= pv_lag:
                        emit_pv()
                    # PE fill work while ScalarE runs the exps
                    if t == 0:
                        for _ in range(_TUNE["fill0_pops"]):
                            if fill0:
                                fill0.pop(0)()
                    else:
                        if t + 1 < NH and kc in kt_slots:
                            ci = kt_slots[kc]
                            emit_ktproj(t + 1, *kpN[ci])
                        if t + 2 < NH and kc == qt_slot:
                            emit_qtproj(t + 2)
                        if t in oa_pair and kc in oa_slots[t]:
                            emit_outA(oa_slots[t][kc])
                        if fillB and t >= 1:
                            fillB.pop(0)()
                        if (
                            t == NH - 1
                            and _TUNE["fillC_from"] <= kc < NKV
                            and fillC
                        ):
                            fillC.pop(0)()
                    if t != NH - 1 and kc >= pv_lag:
                        emit_pv()
                while fill0:
                    fill0.pop(0)()
                if t >= 3:
                    while fillB:
                        fillB.pop(0)()
                if t < NH - 1:
                    mk = make_norm_pe if t == NH - 2 else make_norm_xbar
                    pending_norm = mk(t, o_ps)

            # ---- tail: pair 7 normalize + OT[7] out-proj ------------------
            # The idle PE preloads outpart[mt] into psum banks through an
            # identity matmul that OPENS the accumulation group (start=True);
            # the OT[7] matmul then accumulates and closes it. The final op
            # is a cheap psum->sbuf cast copy split DVE/Act, and stores go
            # out as 3 two-block DMAs + 2 singles.
            t = NH - 1

            tail_ps = [None] * NH

            def preload(mt, pool, tag):
                tail_ps[mt] = pool.tile([128, 512], F32, tag=tag, name="tps")
                nc.tensor.matmul(
                    tail_ps[mt][:], ident[:], outpart[mt][:],
                    start=True, stop=False,
                )

            pre_pool = [
                (sps, "sps"), (proj, "proj"), (sps, "sps"), (proj, "proj"),
                (sps, "sps"), (ops, "ops"), (ops, "ops"), (ops, "ops"),
            ]
            for mt in range(5):
                preload(mt, *pre_pool[mt])

            ot_t = persist.tile([128, LQ], BF16, tag=f"KT{t}", name=f"OT{t}")
            otps = ops.tile([64, 1024], BF16, tag="ops", name="otps")
            nobs = []
            for sub in range(2):
                opv = o_ps[sub].rearrange("p (q c) -> p q c", c=DH + 1)
                rc = nrm.tile([128, 4, 1], F32, tag="rc", name="rc")
                nc.vector.reciprocal(rc[:], opv[:, :, DH : DH + 1])
                nob = nrm.tile([128, 4, DH], BF16, tag="nob", name="nob", bufs=3)
                nc.vector.tensor_tensor(
                    nob[:],
                    opv[:, :, 0:DH],
                    rc[:].broadcast_to([128, 4, DH]),
                    mybir.AluOpType.mult,
                )
                nobs.append(nob)
            for g in range(8):
                nc.tensor.transpose(
                    otps[:, g * 128 : (g + 1) * 128],
                    nobs[g // 4][:, g % 4, :],
                    ident[:],
                )
            # leftover outB1 units: PE is idle while DVE runs part_b; the
            # first rides an ops bank freed by part_a so nothing late blocks
            # it. (GPSIMD cannot touch PSUM on hardware: DVE/Act only here.)
            if fillC:
                fillC.pop(0)(pool=ops, tag="ops")
            while fillC:
                fillC.pop(0)()
            nc.vector.tensor_copy(ot_t[0:64, :], otps[:, 0:512])
            nc.scalar.copy(ot_t[64:128, :], otps[:, 512:1024])
            OT[t] = ot_t
            # ops-bank rotation: leftoverPS<-o_ps[0], pre5<-o_ps[1],
            # pre7<-leftoverPS (WAR on the leftover add), pre6<-otps (WAR on
            # the part_b copies) -- emit in 5,7,6 order to match.
            for mt in (5, 7, 6):
                preload(mt, *pre_pool[mt])

            qs = [nc.sync, nc.scalar]
            cp_eng = [
                nc.vector, nc.scalar, nc.vector, nc.scalar,
                nc.vector, nc.scalar, nc.vector, nc.scalar,
            ]
            # blocks 0-5 store as two-block DMAs; 6 and 7 as singles so the
            # last store's transfer is small and dispatches immediately.
            finb = [
                finpool.tile([128, 2, 512], BF16, tag=f"fin{j}", name=f"fin{j}")
                for j in range(3)
            ] + [
                finpool.tile([128, 1, 512], BF16, tag=f"fin{j}", name=f"fin{j}")
                for j in (3, 4)
            ]
            fin_of = lambda mt: finb[mt // 2][:, mt % 2, :] if mt < 6 else (
                finb[mt - 3][:, 0, :]
            )
            for mt in range(NH):
                ps = tail_ps[mt]
                nc.tensor.matmul(
                    ps[:],
                    wo_bf[NH - 1][:, mt * 128 : (mt + 1) * 128],
                    OT[NH - 1][:],
                    start=False,
                    stop=True,
                )
                fin = fin_of(mt)
                eng = cp_eng[mt]
                with nc.allow_low_precision(reason="bf16 output store"):
                    if eng is nc.scalar:
                        eng.copy(fin, ps[:])
                    else:
                        eng.tensor_copy(fin, ps[:])
                if mt in (1, 3, 5):
                    qs[(mt // 2) % 2].dma_start(
                        out_d.ap()[
                            (mt - 1) * 128 : (mt + 1) * 128, :
                        ].rearrange("(m p) n -> p m n", p=128),
                        finb[mt // 2][:],
                    )
                elif mt >= 6:
                    qs[mt % 2].dma_start(
                        out_d[mt * 128 : (mt + 1) * 128, :], finb[mt - 3][:, 0, :]
                    )

    nc.compile()
    return nc


_NC_CACHE = {}


def get_nc(nkv, mkv=None):
    if mkv is None:
        mkv = nkv * 128
    key = (nkv, mkv)
    if key not in _NC_CACHE:
        _NC_CACHE[key] = build_nc(nkv, mkv)
    return _NC_CACHE[key]


def make_in_maps(query, key_value, kv_mask, Wq, bq, Wk, bk, Wv, bv, Wo, bo):
    f = lambda x: np.ascontiguousarray(np.asarray(x), dtype=np.float32)
    bf = lambda x: np.ascontiguousarray(
        np.asarray(x, dtype=np.float32).astype(ml_dtypes.bfloat16)
    )
    query, key_value = bf(query), bf(key_value)
    mask = np.asarray(kv_mask)
    counts = mask.sum(axis=1).astype(int)
    mkv = max(1, int(counts.max()))
    nkv = -(-mkv // 128)
    lkvc = nkv * 128
    Wo32 = f(Wo)
    bo_eff = (f(bv) @ Wo32 + f(bo)).astype(np.float32)

    def pack_blocks(W):  # [768, 1024] -> [8, 128, 768] per-column-block
        Wb = bf(W).reshape(QD // 128, 128, NH, 128)
        return np.ascontiguousarray(Wb.transpose(2, 1, 0, 3).reshape(NH, 128, QD))

    common = {
        "ident": np.ascontiguousarray(
            np.eye(128, dtype=np.float32).astype(ml_dtypes.bfloat16)
        ),
        "Wq_pk": pack_blocks(Wq),
        "Wk_pk": pack_blocks(Wk),
        "Wv_bf": bf(Wv),
        "Wo_bf": bf(Wo),
    }
    def pack_T(x):  # [L, 768] -> pre-transposed [128, 6*L] (x^T tile layout)
        L = x.shape[0]
        return np.ascontiguousarray(
            x.T.reshape(QD // 128, 128, L).transpose(1, 0, 2).reshape(128, -1)
        )

    # per-partition bias vectors, host-pre-transposed to [128, NBC] and
    # appended to the q transfer as bf16 (exact for 0/-30000 mask values;
    # bq/bk/bo_eff quantization is far below the accuracy budget).
    bias_head = np.concatenate([f(bq), f(bk), bo_eff])  # [3*1024]
    in_maps = []
    for b in range(B):
        m = dict(common)
        n = int(counts[b])
        kv_c = np.zeros((lkvc, QD), dtype=ml_dtypes.bfloat16)
        kv_c[:n] = key_value[b][mask[b]]
        mb = np.full((lkvc,), MASK_NEG, dtype=np.float32)
        mb[:n] = 0.0
        bias_cat = np.concatenate([bias_head, mb]).reshape(-1, 128)  # [NBC,128]
        q_pk = pack_T(query[b])  # [128, 6*512]
        m["q_pk"] = np.ascontiguousarray(
            np.concatenate(
                [q_pk, bias_cat.T.astype(ml_dtypes.bfloat16)], axis=1
            )
        )
        m["kv_pk"] = pack_T(kv_c)
        in_maps.append(m)
    return in_maps, nkv, mkv


def kernel(**inputs) -> np.ndarray:
    in_maps, nkv, mkv = make_in_maps(**inputs)
    nc = get_nc(nkv, mkv)
    res = run_bass_kernel_spmd(nc, in_maps, core_ids=list(range(B)))
    out = np.stack([res.results[i]["out"].T for i in range(B)])
    return np.ascontiguousarray(out.astype(np.float32))


# revision 94
# speedup vs baseline: 1.0026x; 1.0026x over previous
"""Trainium2 Bass kernel for MultiHeadCrossAttention.

Problem shapes (hardcoded; see module constants):
  query      [8, 512, 768] f32
  key_value  [8, 2048, 768] f32
  kv_mask    [8, 2048] bool
  Wq/Wk/Wv   [768, 1024] f32, Wo [1024, 1024] f32, biases [1024] f32

Sharding: pure data-parallel -- batch element b runs on core b (8 cores, no
collectives). Each core computes the full attention stack for its batch
element and writes out^T [1024, 512]; the host transposes and stacks.

Host-side prep (as the 140.2us baseline): kv-mask compaction to NKV*128
rows, bf16 weights/activations, kv_mask folded to an additive bias, bv
folded into bo, q/kv pre-transposed, Wq/Wk packed per-column-block.

v2 changes over the 140.2us baseline (cost-model time now ~133.3us):
  - All per-partition bias vectors (bq|bk|bo_eff|mask) are pre-TRANSPOSED
    on the host and appended to the q DMA as bf16 columns -- no separate
    bias DMA, no identity-matrix dependency, no PE transpose at startup
    (one DVE copy widens them to f32 for the tensor_scalar ops).
  - kv^T arrives in independent piece TILES (one DMA each) so the
    transfers pipeline with no WAW serialization, and the load order makes
    every pair-0 input an early item on its queue (gpsimd wq0+wk0 via
    SWDGE, scalar kv pieces, sync q+bias then a 256-col Wv slice for
    heads 0-3). S(0,0) fires at ~8us instead of ~11.5us; V projection is
    emitted in 4-head column groups so pair 0 only waits the small Wv
    slice. K-proj pieces align to the kv piece tiles.
  - K-proj is trimmed to the exact compacted kv count MKV (KT pad cols are
    memset once; their scores die under the -30000 mask bias).
  - O^T transposes for pairs 0-5 go to the DMA crossbar (dma_start_transpose
    on sync) instead of the PE: part_a writes a [128,qb,sub,dh] interleave,
    4 [128,128] xbar tiles write OT directly (no otps PSUM, no part_b).
    Pair 6 keeps the PE path (its OT feeds outB1 too soon for the xbar
    latency); pair 7's normalize runs inline at the tail.
  - Tail: while the pair-7 normalize runs on DVE/Act, the idle PE preloads
    outpart[mt] into psum banks via identity matmuls that OPEN the
    accumulation groups (start=True); the OT[7] matmuls then accumulate
    onto them (start=False, stop=True), so the final op per block is a
    cheap psum->sbuf cast copy alternating DVE/Act, and the output leaves
    as 4 two-block DMAs alternating sync/scalar. (GPSIMD cannot touch
    PSUM on hardware, and matmul-accumulate over engine-written PSUM
    mis-lowers on hardware -- hence the identity-matmul preloads.)
  - The cost model locks each matmul's p-state price when it enters the
    PE exec queue, so the schedule keeps ~30 dummy warmup matmuls and the
    _TUNE knobs were picked by timeline-sim sweep with numerics re-checked
    in CoreSim (a faster schedule that drops work is worthless).
"""

import numpy as np
import ml_dtypes

import concourse.bass as bass
import concourse.bacc as bacc
import concourse.mybir as mybir
import concourse.tile as tile
from concourse.bass_utils import run_bass_kernel_spmd

dt = mybir.dt
AF = mybir.ActivationFunctionType

B = 8
LQ = 512
LKV = 2048
QD = 768
HID = 1024
H = 16
DH = 64
SCALE = DH**-0.5
MASK_NEG = -30000.0

F32 = dt.float32
BF16 = dt.bfloat16

NQT = QD // 128  # 6 feature tiles
NH = HID // 128  # 8 hidden tiles

# schedule tuning knobs (defaults chosen by timeline-sim sweep)
_TUNE = {
    "warmup": 30,        # dummy p-state ramp matmuls
    "kv_cuts": (0, 256, 512, 768),  # kv piece boundaries
    "pv_lag0": 1,        # extra PV lag for pair 0
    "fill0_pops": 2,     # pair-0 fill units consumed per kc slot
    "qt_slot": 1,        # slot for the t+2 Q projection
    "fillC_from": 2,     # first pair-7 slot consuming outB1 units
}


def build_nc(NKV, MKV):
    """Per-core kernel; compacted kv capacity NKV*128 rows, exact max
    unmasked count MKV (K-proj only computes cols < MKV)."""
    LKVC = NKV * 128
    MKV = min(MKV, LKVC)
    NBC = 3 * NH + NKV  # bias cols appended to q: bq|bk|bo_eff|mask

    # kv arrives in 3 independent piece TILES (separate tiles so the DMAs
    # pipeline with no WAW serialization); K-proj pieces align to them.
    kv_cuts = [c for c in _TUNE["kv_cuts"] if c < LKVC] + [LKVC]
    kv_loads = list(zip(kv_cuts[:-1], kv_cuts[1:]))

    def k_pieces():
        # <=512-col pieces, each inside one kv piece tile
        pieces = []
        for p0, p1 in kv_loads:
            c = p0
            hi = min(p1, MKV)
            while c < hi:
                pieces.append((c, min(c + 512, hi)))
                c = min(c + 512, hi)
        return pieces

    nc = bacc.Bacc("TRN2", target_bir_lowering=False, debug=False)

    q_d = nc.dram_tensor(
        "q_pk", [128, NQT * LQ + NBC], BF16, kind="ExternalInput"
    )
    kv_d = nc.dram_tensor("kv_pk", [128, NQT * LKVC], BF16, kind="ExternalInput")
    wq_d = nc.dram_tensor("Wq_pk", [NH, 128, QD], BF16, kind="ExternalInput")
    wk_d = nc.dram_tensor("Wk_pk", [NH, 128, QD], BF16, kind="ExternalInput")
    wv_d = nc.dram_tensor("Wv_bf", [QD, HID], BF16, kind="ExternalInput")
    wo_d = nc.dram_tensor("Wo_bf", [HID, HID], BF16, kind="ExternalInput")
    id_d = nc.dram_tensor("ident", [128, 128], BF16, kind="ExternalInput")
    out_d = nc.dram_tensor("out", [HID, LQ], BF16, kind="ExternalOutput")

    with tile.TileContext(nc) as tc:
        with (
            tc.tile_pool(name="persist", bufs=1) as persist,
            tc.tile_pool(name="ppool", bufs=3) as ppool,
            tc.tile_pool(name="nrm", bufs=2) as nrm,
            tc.tile_pool(name="finpool", bufs=8) as finpool,
            # PSUM: 8 banks of 2KB/partition total.
            tc.tile_pool(name="sps", bufs=3, space="PSUM") as sps,    # 3 banks
            tc.tile_pool(name="proj", bufs=2, space="PSUM") as proj,  # 2 banks
            tc.tile_pool(name="ops", bufs=3, space="PSUM") as ops,    # 3 banks
        ):
            qb_t = persist.tile([128, NQT * LQ + NBC], BF16, tag="qTb", name="qTb")

            def qTb(kt):
                return qb_t[:, kt * LQ : (kt + 1) * LQ]
            # bias columns ride the q DMA as bf16; widen to f32 once (the
            # tensor_scalar ops require an f32 scalar operand)
            bias_bf = qb_t[:, NQT * LQ :]
            bc32 = persist.tile([128, NBC], F32, tag="bc32", name="bc32")
            bq_sb = bc32[:, 0:NH]
            bk_sb = bc32[:, NH : 2 * NH]
            boe_sb = bc32[:, 2 * NH : 3 * NH]
            mb_sb = bc32[:, 3 * NH : 3 * NH + NKV]
            kvt = [
                persist.tile(
                    [128, NQT, c1 - c0], BF16, tag=f"kvt{i}", name=f"kvt{i}"
                )
                for i, (c0, c1) in enumerate(kv_loads)
            ]

            def kv_ap(kt, c0, c1):
                """SBUF AP for kv^T cols [c0, c1) (must lie in one piece)."""
                for (p0, p1), tile_ in zip(kv_loads, kvt):
                    if c0 >= p0 and c1 <= p1:
                        return tile_[:, kt, c0 - p0 : c1 - p0]
                raise AssertionError(f"kv range {c0}:{c1} spans pieces")

            ident = persist.tile([128, 128], BF16, tag="ident")
            wqb = [None] * NH
            wkb = [None] * NH
            # Wv column groups: wvg[0] = hid cols 0:256 (heads 0-3) from its
            # own small DMA; cols 256:1024 (heads 4-15) in one tile.
            wv0_t = persist.tile([128, NQT, 256], BF16, tag="wv0", name="wv0")
            wvr_t = persist.tile([128, NQT, 768], BF16, tag="wvr", name="wvr")
            wo_bf = []

            def load_wblock(dst_list, src_d, b, eng):
                wt = persist.tile(
                    [128, NQT, 128], BF16, tag=f"w{id(src_d)}b{b}",
                    name=f"wb{b}",
                )
                eng.dma_start(wt.rearrange("p kt c -> p (kt c)"), src_d[b])
                dst_list[b] = wt

            def load_wv_cols(dst, c0, c1, eng):
                eng.dma_start(
                    dst[:],
                    wv_d.ap()[:, c0:c1].rearrange("(kt p) c -> p kt c", p=128),
                )

            def load_kv_piece(i, eng):
                c0, c1 = kv_loads[i]
                eng.dma_start(
                    kvt[i][:],
                    kv_d.ap().rearrange("p (f n) -> p f n", n=LKVC)[:, :, c0:c1],
                )

            # ---- staging DMAs ---------------------------------------------
            # All transfers serialize on the shared DMA-engine device in
            # ~emission order (independent tiles pipeline; same-tile pieces
            # would serialize on WAW sems). Priority: q+bias, kv piece 0,
            # wk0/wq0 (SWDGE, parallel gen), kv piece 1, wv heads 0-3, wq1,
            # kv piece 2, wk1, wv rest, then the remaining weights.
            load_wblock(wqb, wq_d, 0, nc.gpsimd)
            load_kv_piece(0, nc.scalar)
            nc.sync.dma_start(qb_t[:], q_d[:])
            load_wblock(wkb, wk_d, 0, nc.gpsimd)
            for i in range(1, len(kv_loads)):
                load_kv_piece(i, nc.scalar)
            load_wv_cols(wv0_t, 0, 256, nc.sync)
            load_wblock(wqb, wq_d, 1, nc.gpsimd)
            load_wblock(wkb, wk_d, 1, nc.gpsimd)
            load_wv_cols(wvr_t, 256, 1024, nc.sync)
            for b in range(2, NH):
                load_wblock(wkb, wk_d, b, nc.scalar)
                load_wblock(wqb, wq_d, b, nc.gpsimd)
            nc.gpsimd.dma_start(ident[:], id_d[:])
            for kt in range(NH):
                wt = persist.tile([128, HID], BF16, tag=f"wo{kt}", name=f"wo{kt}")
                nc.sync.dma_start(wt[:], wo_d[kt * 128 : (kt + 1) * 128, :])
                wo_bf.append(wt)

            # PE p-state warmup on dummy matmuls of a memset scratch tile
            # (proven recipe: ~3.2us of continuous dummy work from t~1us
            # leaves every later matmul at full clock).
            wsc = persist.tile([128, 128], BF16, tag="wsc", name="wsc")
            nc.vector.memset(wsc[:], 0.0)
            wrm = None
            for i in range(_TUNE["warmup"]):
                if i % 4 == 0:
                    wrm = sps.tile([128, 512], F32, tag="sps", name="wrm")
                nc.tensor.matmul(
                    wrm[:, (i % 4) * 128 : (i % 4 + 1) * 128],
                    wsc[:],
                    wsc[:],
                    start=True,
                    stop=True,
                )

            # KT pad columns [MKV:LKVC] are read by the last kv chunk's S
            # matmuls but never written by the trimmed K-proj; zero once.
            KT = [
                persist.tile([128, LKVC], BF16, tag=f"KT{t}", name=f"KT{t}")
                for t in range(NH)
            ]
            if MKV < LKVC:
                for t in range(NH):
                    nc.vector.memset(KT[t][:, MKV:LKVC], 0.0)
            nc.vector.tensor_copy(bc32[:], bias_bf)

            # ---- Q^T projection tiles [128, 512] bf16 ---------------------
            QT = [None] * NH

            def emit_qtproj(mt):
                ps = proj.tile([128, 512], F32, tag="proj", name="ps")
                for kt in range(NQT):
                    nc.tensor.matmul(
                        ps[:],
                        wqb[mt][:, kt, :],
                        qTb(kt),
                        start=(kt == 0),
                        stop=(kt == NQT - 1),
                    )
                qt_t = persist.tile([128, LQ], BF16, tag=f"QT{mt}", name=f"QT{mt}")
                nc.vector.tensor_scalar_add(qt_t[:], ps[:], bq_sb[:, mt : mt + 1])
                QT[mt] = qt_t

            # ---- V projection in 4-head column groups ---------------------
            # V_il[lt] is [128, 16, 65] head-interleaved with a ones column
            # (PV's extra column = softmax denominator). Group g covers
            # heads 4g..4g+3 (pairs 2g, 2g+1).
            V_il = [None] * NKV

            def emit_vproj_g(lt, g):
                if V_il[lt] is None:
                    vt = persist.tile(
                        [128, H, DH + 1], BF16, tag=f"V{lt}", name=f"V{lt}"
                    )
                    nc.vector.memset(vt[:, :, DH], 1.0)
                    V_il[lt] = vt
                vt = V_il[lt]
                wsrc = wv0_t if g == 0 else wvr_t
                coff = 0 if g == 0 else (g - 1) * 256
                ps = proj.tile([128, 512], F32, tag="proj", name="ps")
                for kt in range(NQT):
                    nc.tensor.matmul(
                        ps[:, 0:256],
                        kv_ap(kt, lt * 128, (lt + 1) * 128),
                        wsrc[:, kt, coff : coff + 256],
                        start=(kt == 0),
                        stop=(kt == NQT - 1),
                    )
                nc.vector.tensor_copy(
                    vt[:, 4 * g : 4 * g + 4, 0:DH],
                    ps[:, 0:256].rearrange("p (h d) -> p h d", d=DH),
                )

            # ---- K^T projection (trimmed to MKV cols) ---------------------
            def emit_ktproj(t, c0, c1):
                w = c1 - c0
                ps = proj.tile([128, 512], F32, tag="proj", name="ps")
                for kt in range(NQT):
                    nc.tensor.matmul(
                        ps[:, 0:w],
                        wkb[t][:, kt, :],
                        kv_ap(kt, c0, c1),
                        start=(kt == 0),
                        stop=(kt == NQT - 1),
                    )
                nc.vector.tensor_scalar_add(
                    KT[t][:, c0:c1], ps[:, 0:w], bk_sb[:, t : t + 1]
                )

            kp0 = k_pieces()
            kpN = k_pieces()
            emit_qtproj(0)
            emit_ktproj(0, *kp0[0])

            # Pair-0 PE fill queue in expected data-arrival order: each
            # K-proj piece, the V group-0 chunks its kv cols enable, then
            # pair-1 projections.
            fill0 = []

            def mk_v(j, g):
                return lambda: emit_vproj_g(j, g)

            nv = 0
            for pi, (c0, c1) in enumerate(kp0[1:], start=1):
                fill0.append(lambda c0=c0, c1=c1: emit_ktproj(0, c0, c1))
                if pi == 1 and NH > 1:
                    fill0.append(lambda: emit_qtproj(1))
                vmax = min(-(-c1 // 128), NKV)
                while nv < vmax:
                    fill0.append(mk_v(nv, 0))
                    nv += 1
            while nv < NKV:
                fill0.append(mk_v(nv, 0))
                nv += 1
            if NH > 2:
                fill0.append(lambda: emit_qtproj(2))
            for c0, c1 in kpN:
                if NH > 1:
                    fill0.append(lambda c0=c0, c1=c1: emit_ktproj(1, c0, c1))
            # V groups 1-3 (heads 4-15): fill for pairs 1-3; group g first
            # needed by pair 2g slot 1.
            fillB = [mk_v(j, g) for g in range(1, 4) for j in range(NKV)]

            # ---- attention per head pair ---------------------------------
            outpart = [None] * NH

            def emit_outA(mt):
                ps = proj.tile([128, 512], F32, tag="proj", name="ps")
                for kt in range(4):
                    nc.tensor.matmul(
                        ps[:],
                        wo_bf[kt][:, mt * 128 : (mt + 1) * 128],
                        OT[kt][:],
                        start=(kt == 0),
                        stop=(kt == 3),
                    )
                op_t = persist.tile(
                    [128, 512], BF16, tag=f"outpart{mt}", name=f"outpart{mt}"
                )
                with nc.allow_low_precision(reason="bf16 out partial"):
                    nc.vector.tensor_scalar_add(
                        op_t[:], ps[:], boe_sb[:, mt : mt + 1]
                    )
                outpart[mt] = op_t

            kt_slots = {}
            for ci in range(len(kpN)):
                kt_slots[max(1, ((ci + 1) * NKV) // (len(kpN) + 1))] = ci
            assert len(kt_slots) == len(kpN), "K-proj piece slot collision"
            qt_slot = min(_TUNE["qt_slot"], NKV - 1)
            # outA: OT[3]'s xbar lands ~2 slots into pair 4.
            oa_pair = {4: [0, 1], 5: [2, 3], 6: [4, 5, 6, 7]}
            oa_slots = {
                4: {min(5, NKV - 2): 0, min(8, NKV - 1): 1},
                5: {min(3, NKV - 2): 2, min(6, NKV - 1): 3},
                6: {
                    min(2, NKV - 4): 4,
                    min(4, NKV - 3): 5,
                    min(6, NKV - 2): 6,
                    min(8, NKV - 1): 7,
                },
            }

            def emit_outB1(mt, eng=None, pool=None, tag="proj"):
                ps = (pool or proj).tile([128, 512], F32, tag=tag, name="ps")
                for kt in range(4, NH - 1):
                    nc.tensor.matmul(
                        ps[:],
                        wo_bf[kt][:, mt * 128 : (mt + 1) * 128],
                        OT[kt][:],
                        start=(kt == 4),
                        stop=(kt == NH - 2),
                    )
                with nc.allow_low_precision(reason="bf16 out partial"):
                    (eng or nc.vector).tensor_tensor(
                        outpart[mt][:], outpart[mt][:], ps[:],
                        mybir.AluOpType.add,
                    )

            fillC = [
                lambda eng=None, mt=mt, **kw: emit_outB1(mt, eng, **kw)
                for mt in range(NH)
            ]

            def make_norm_xbar(t, o_ps):
                """Deferred normalize via the DMA crossbar (sync queue)."""
                ot_t = persist.tile([128, LQ], BF16, tag=f"KT{t}", name=f"OT{t}")
                nobx = nrm.tile(
                    [128, 4, 2, DH], BF16, tag="nobx", name="nobx", bufs=2
                )

                def part_a():
                    for sub in range(2):
                        opv = o_ps[sub].rearrange("p (q c) -> p q c", c=DH + 1)
                        rc = nrm.tile([128, 4, 1], F32, tag="rc", name="rc")
                        nc.vector.reciprocal(rc[:], opv[:, :, DH : DH + 1])
                        nc.vector.tensor_tensor(
                            nobx[:, :, sub, :],
                            opv[:, :, 0:DH],
                            rc[:].broadcast_to([128, 4, DH]),
                            mybir.AluOpType.mult,
                        )

                def part_b():
                    for qb in range(4):
                        nc.sync.dma_start_transpose(
                            ot_t[:, qb * 128 : (qb + 1) * 128],
                            nobx[:, qb, :, :],
                        )
                    OT[t] = ot_t

                return [part_a, part_b]

            def make_norm_pe(t, o_ps):
                """PE-transpose normalize for pair 6 (OT[6] is consumed too
                soon into pair 7 for the xbar latency)."""
                ot_t = persist.tile([128, LQ], BF16, tag=f"KT{t}", name=f"OT{t}")
                otps = ops.tile([64, 1024], BF16, tag="ops", name="otps")

                def part_a():
                    nobs = []
                    for sub in range(2):
                        opv = o_ps[sub].rearrange("p (q c) -> p q c", c=DH + 1)
                        rc = nrm.tile([128, 4, 1], F32, tag="rc", name="rc")
                        nc.vector.reciprocal(rc[:], opv[:, :, DH : DH + 1])
                        nob = nrm.tile(
                            [128, 4, DH], BF16, tag="nob", name="nob", bufs=3
                        )
                        nc.vector.tensor_tensor(
                            nob[:],
                            opv[:, :, 0:DH],
                            rc[:].broadcast_to([128, 4, DH]),
                            mybir.AluOpType.mult,
                        )
                        nobs.append(nob)
                    for g in range(8):
                        nc.tensor.transpose(
                            otps[:, g * 128 : (g + 1) * 128],
                            nobs[g // 4][:, g % 4, :],
                            ident[:],
                        )

                def part_b():
                    for sub in range(2):
                        nc.vector.tensor_copy(
                            ot_t[sub * 64 : sub * 64 + 64, :],
                            otps[:, sub * 512 : (sub + 1) * 512],
                        )
                    OT[t] = ot_t

                return [part_a, part_b]

            OT = [None] * NH
            pending_norm = []
            for t in range(NH):
                o_ps = [
                    ops.tile([128, 4 * (DH + 1)], F32, tag="ops", name="o_ps")
                    for _ in range(2)
                ]
                # pair 0's V group lands ~1 slot later than its first PV
                # would like; lag the PV stream one extra slot there.
                pv_lag = _TUNE["pv_lag0"] if t == 0 else 1
                p_hist = []
                for kc in range(NKV + pv_lag):
                    if kc < NKV:
                        p_cur = [None, None]
                        for sub in range(2):
                            off = sub * 64
                            s = sps.tile([128, 512], F32, tag="sps", name="s")
                            nc.tensor.matmul(
                                s[:],
                                KT[t][off : off + 64, kc * 128 : (kc + 1) * 128],
                                QT[t][off : off + 64, :],
                                start=True,
                                stop=True,
                            )
                            p = ppool.tile(
                                [128, 512], BF16, tag="p", name="p", bufs=6
                            )
                            nc.scalar.activation(
                                p[:], s[:], AF.Exp,
                                bias=mb_sb[:, kc : kc + 1], scale=SCALE,
                            )
                            p_cur[sub] = p
                        p_hist.append(p_cur)
                    if kc < len(pending_norm):
                        pending_norm[kc]()

                    def emit_pv():
                        kcp = kc - pv_lag
                        pp = p_hist[kcp]
                        for sub in range(2):
                            for qb in range(4):
                                nc.tensor.matmul(
                                    o_ps[sub][:, qb * 65 : qb * 65 + 65],
                                    pp[sub][:, qb * 128 : (qb + 1) * 128],
                                    V_il[kcp][:, 2 * t + sub, :],
                                    start=(kcp == 0 and qb == 0),
                                    stop=(kcp == NKV - 1 and qb == 3),
                                )

                    # For the LAST pair the S/PV chain goes AHEAD of the
                    # fills: fills otherwise delay the final S matmuls in
                    # the in-order PE queue, starving the Act exp stream
                    # whose last exp gates the whole tail.
                    if t == NH - 1 and kc >= pv_lag:
                        emit_pv()
                    # PE fill work while ScalarE runs the exps
                    if t == 0:
                        for _ in range(_TUNE["fill0_pops"]):
                            if fill0:
                                fill0.pop(0)()
                    else:
                        if t + 1 < NH and kc in kt_slots:
                            ci = kt_slots[kc]
                            emit_ktproj(t + 1, *kpN[ci])
                        if t + 2 < NH and kc == qt_slot:
                            emit_qtproj(t + 2)
                        if t in oa_pair and kc in oa_slots[t]:
                            emit_outA(oa_slots[t][kc])
                        if fillB and t >= 1:
                            fillB.pop(0)()
                        if (
                            t == NH - 1
                            and _TUNE["fillC_from"] <= kc < NKV
                            and fillC
                        ):
                            fillC.pop(0)()
                    if t != NH - 1 and kc >= pv_lag:
                        emit_pv()
                while fill0:
                    fill0.pop(0)()
                if t >= 3:
                    while fillB:
                        fillB.pop(0)()
                if t < NH - 1:
                    mk = make_norm_pe if t == NH - 2 else make_norm_xbar
                    pending_norm = mk(t, o_ps)

            # ---- tail: pair 7 normalize + OT[7] out-proj ------------------
            # The idle PE preloads outpart[mt] into psum banks through an
            # identity matmul that OPENS the accumulation group (start=True);
            # the OT[7] matmul then accumulates and closes it. The final op
            # is a cheap psum->sbuf cast copy split DVE/Act, and stores go
            # out as 3 two-block DMAs + 2 singles.
            t = NH - 1

            tail_ps = [None] * NH

            def preload(mt, pool, tag):
                tail_ps[mt] = pool.tile([128, 512], F32, tag=tag, name="tps")
                nc.tensor.matmul(
                    tail_ps[mt][:], ident[:], outpart[mt][:],
                    start=True, stop=False,
                )

            pre_pool = [
                (sps, "sps"), (proj, "proj"), (sps, "sps"), (proj, "proj"),
                (sps, "sps"), (ops, "ops"), (ops, "ops"), (ops, "ops"),
            ]
            for mt in range(5):
                preload(mt, *pre_pool[mt])

            ot_t = persist.tile([128, LQ], BF16, tag=f"KT{t}", name=f"OT{t}")
            otps = ops.tile([64, 1024], BF16, tag="ops", name="otps")
            nobs = []
            for sub in range(2):
                opv = o_ps[sub].rearrange("p (q c) -> p q c", c=DH + 1)
                rc = nrm.tile([128, 4, 1], F32, tag="rc", name="rc")
                nc.vector.reciprocal(rc[:], opv[:, :, DH : DH + 1])
                nob = nrm.tile([128, 4, DH], BF16, tag="nob", name="nob", bufs=3)
                nc.vector.tensor_tensor(
                    nob[:],
                    opv[:, :, 0:DH],
                    rc[:].broadcast_to([128, 4, DH]),
                    mybir.AluOpType.mult,
                )
                nobs.append(nob)
            for g in range(8):
                nc.tensor.transpose(
                    otps[:, g * 128 : (g + 1) * 128],
                    nobs[g // 4][:, g % 4, :],
                    ident[:],
                )
            # leftover outB1 units: PE is idle while DVE runs part_b; the
            # first rides an ops bank freed by part_a so nothing late blocks
            # it. (GPSIMD cannot touch PSUM on hardware: DVE/Act only here.)
            if fillC:
                fillC.pop(0)(pool=ops, tag="ops")
            while fillC:
                fillC.pop(0)()
            nc.vector.tensor_copy(ot_t[0:64, :], otps[:, 0:512])
            nc.scalar.copy(ot_t[64:128, :], otps[:, 512:1024])
            OT[t] = ot_t
            # ops-bank rotation: leftoverPS<-o_ps[0], pre5<-o_ps[1],
            # pre7<-leftoverPS (WAR on the leftover add), pre6<-otps (WAR on
            # the part_b copies) -- emit in 5,7,6 order to match.
            for mt in (5, 7, 6):
                preload(mt, *pre_pool[mt])

            qs = [nc.sync, nc.scalar]
            cp_eng = [
                nc.vector, nc.scalar, nc.vector, nc.scalar,
                nc.vector, nc.scalar, nc.vector, nc.scalar,
            ]
            # blocks 0-5 store as two-block DMAs; 6 and 7 as singles so the
            # last store's transfer is small and dispatches immediately.
            finb = [
                finpool.tile([128, 2, 512], BF16, tag=f"fin{j}", name=f"fin{j}")
                for j in range(3)
            ] + [
                finpool.tile([128, 1, 512], BF16, tag=f"fin{j}", name=f"fin{j}")
                for j in (3, 4)
            ]
            fin_of = lambda mt: finb[mt // 2][:, mt % 2, :] if mt < 6 else (
                finb[mt - 3][:, 0, :]
            )
            for mt in range(NH):
                ps = tail_ps[mt]
                nc.tensor.matmul(
                    ps[:],
                    wo_bf[NH - 1][:, mt * 128 : (mt + 1) * 128],
                    OT[NH - 1][:],
                    start=False,
                    stop=True,
                )
                fin = fin_of(mt)
                eng = cp_eng[mt]
                with nc.allow_low_precision(reason="bf16 output store"):
                    if eng is nc.scalar:
                        eng.copy(fin, ps[:])
                    else:
                        eng.tensor_copy(fin, ps[:])
                if mt in (1, 3, 5):
                    qs[(mt // 2) % 2].dma_start(
                        out_d.ap()[
                            (mt - 1) * 128 : (mt + 1) * 128, :
                        ].rearrange("(m p) n -> p m n", p=128),
                        finb[mt // 2][:],
                    )
                elif mt >= 6:
                    qs[mt % 2].dma_start(
                        out_d[mt * 128 : (mt + 1) * 128, :], finb[mt - 3][:, 0, :]
                    )

    nc.compile()
    return nc


_NC_CACHE = {}


def get_nc(nkv, mkv=None):
    if mkv is None:
        mkv = nkv * 128
    key = (nkv, mkv)
    if key not in _NC_CACHE:
        _NC_CACHE[key] = build_nc(nkv, mkv)
    return _NC_CACHE[key]


def make_in_maps(query, key_value, kv_mask, Wq, bq, Wk, bk, Wv, bv, Wo, bo):
    f = lambda x: np.ascontiguousarray(np.asarray(x), dtype=np.float32)
    bf = lambda x: np.ascontiguousarray(
        np.asarray(x, dtype=np.float32).astype(ml_dtypes.bfloat16)
    )
    query, key_value = bf(query), bf(key_value)
    mask = np.asarray(kv_mask)
    counts = mask.sum(axis=1).astype(int)
    mkv = max(1, int(counts.max()))
    nkv = -(-mkv // 128)
    lkvc = nkv * 128
    Wo32 = f(Wo)
    bo_eff = (f(bv) @ Wo32 + f(bo)).astype(np.float32)

    def pack_blocks(W):  # [768, 1024] -> [8, 128, 768] per-column-block
        Wb = bf(W).reshape(QD // 128, 128, NH, 128)
        return np.ascontiguousarray(Wb.transpose(2, 1, 0, 3).reshape(NH, 128, QD))

    common = {
        "ident": np.ascontiguousarray(
            np.eye(128, dtype=np.float32).astype(ml_dtypes.bfloat16)
        ),
        "Wq_pk": pack_blocks(Wq),
        "Wk_pk": pack_blocks(Wk),
        "Wv_bf": bf(Wv),
        "Wo_bf": bf(Wo),
    }
    def pack_T(x):  # [L, 768] -> pre-transposed [128, 6*L] (x^T tile layout)
        L = x.shape[0]
        return np.ascontiguousarray(
            x.T.reshape(QD // 128, 128, L).transpose(1, 0, 2).reshape(128, -1)
        )

    # per-partition bias vectors, host-pre-transposed to [128, NBC] and
    # appended to the q transfer as bf16 (exact for 0/-30000 mask values;
    # bq/bk/bo_eff quantization is far below the accuracy budget).
    bias_head = np.concatenate([f(bq), f(bk), bo_eff])  # [3*1024]
    in_maps = []
    for b in range(B):
        m = dict(common)
        n = int(counts[b])
        kv_c = np.zeros((lkvc, QD), dtype=ml_dtypes.bfloat16)
        kv_c[:n] = key_value[b][mask[b]]
        mb = np.full((lkvc,), MASK_NEG, dtype=np.float32)
        mb[:n] = 0.0
        bias_cat = np.concatenate([bias_head, mb]).reshape(-1, 128)  # [NBC,128]
        q_pk = pack_T(query[b])  # [128, 6*512]
        m["q_pk"] = np.ascontiguousarray(
            np.concatenate(
                [q_pk, bias_cat.T.astype(ml_dtypes.bfloat16)], axis=1
            )
        )
        m["kv_pk"] = pack_T(kv_c)
        in_maps.append(m)
    return in_maps, nkv, mkv


def kernel(**inputs) -> np.ndarray:
    in_maps, nkv, mkv = make_in_maps(**inputs)
    nc = get_nc(nkv, mkv)
    res = run_bass_kernel_spmd(nc, in_maps, core_ids=list(range(B)))
    out = np.stack([res.results[i]["out"].T for i in range(B)])
    return np.ascontiguousarray(out.astype(np.float32))
